# revision 1
# baseline (speedup 1.0000x reference)
"""Fused FP8-block-quantized MLP (silu(x@w1.T) * (x@w3.T)) @ w2.T on 8 trn2 cores.

Sharding: data-parallel over tokens. Each core gets T/8 = 512 tokens and the
full (dequantized, bf16) weights; there are no collectives. Host-side prep
dequantizes the block-quantized weights, casts to bf16, and lays tensors out
partition-major so every device DMA is one large contiguous transfer.

Device kernel per core (all matmuls bf16, fp32 PSUM accumulation):
  phase A: for each 128-row block fb of F: g.T/u.T [128f, 512t] accumulated
           over 16 k-blocks of H; silu+copy on ACT, mul on DVE -> fusedT
           kept in SBUF.
  phase B: out [512t, 2048h] = fusedT.T @ w2.T, streaming w2 column blocks,
           accumulating over the 56 f-blocks in PSUM.
"""

import sys

import numpy as np

_REPO = "/opt/trn_rl_repo"
if _REPO not in sys.path:
    sys.path.insert(0, _REPO)

T, H, F = 4096, 2048, 7168
NCORES = 8
TC = T // NCORES      # 512 tokens per core
KB = H // 128         # 16 contraction blocks for matmul 1/3
FB = F // 128         # 56 f blocks
FB2 = FB // 2         # w2 blocks are streamed in pairs
HCOLS = H // 512      # 4 output column groups
TB = TC // 128        # 4 token blocks

_CACHE = {}


def _build_program():
    import concourse.mybir as mybir
    from concourse import bacc
    from concourse.tile import TileContext

    bf16 = mybir.dt.bfloat16
    f32 = mybir.dt.float32

    # Bacc (not bass.Bass): its finalize() runs generate_event_semaphores,
    # which splits multi-wait sync_info into EventSemaphore instructions —
    # TRN2 instructions physically carry at most one sem wait.
    nc = bacc.Bacc()
    # All inputs are laid out partition-major on the host so each DMA below
    # is a single large transfer with contiguous per-partition rows.
    xt_d = nc.declare_dram_parameter("xt", [128, KB, TC], bf16, isOutput=False)
    w13_d = nc.declare_dram_parameter(
        "w13p", [FB, 128, 2, H], bf16, isOutput=False
    )
    w2_d = nc.declare_dram_parameter(
        "w2p", [HCOLS, FB2, 128, 2, 512], bf16, isOutput=False
    )
    out_d = nc.declare_dram_parameter("out", [TC, H], f32, isOutput=True)

    with TileContext(nc) as tc:
        with (
            tc.tile_pool(name="xpool", bufs=1) as xpool,
            tc.tile_pool(name="wpool", bufs=2) as wpool,
            tc.tile_pool(name="w2pool", bufs=8) as w2pool,
            tc.tile_pool(name="sgpool", bufs=3) as sgpool,
            tc.tile_pool(name="upool", bufs=3) as upool,
            tc.tile_pool(name="fpool", bufs=FB) as fpool,
            tc.tile_pool(name="opool", bufs=HCOLS * TB) as opool,
            tc.tile_pool(name="psg", bufs=2, space="PSUM") as psg,
            tc.tile_pool(name="psu", bufs=2, space="PSUM") as psu,
            tc.tile_pool(name="psb", bufs=4, space="PSUM") as psb,
        ):
            xtile = xpool.tile([128, KB, TC], bf16)

            fused = []
            for fb in range(FB):
                w13t = wpool.tile([128, 2, H], bf16, tag="w13t")
                if fb == 0:
                    # Quarter the startup loads so the first k-blocks of
                    # matmuls start after ~0.75MB instead of the full 3MB,
                    # with arrivals still dense enough to keep HAM warm.
                    kq, hq = KB // 4, H // 4
                    for q in range(4):
                        nc.sync.dma_start(
                            out=xtile[:, q * kq : (q + 1) * kq, :],
                            in_=xt_d[:, q * kq : (q + 1) * kq, :],
                        )
                        nc.sync.dma_start(
                            out=w13t[:, :, q * hq : (q + 1) * hq],
                            in_=w13_d[fb][:, :, q * hq : (q + 1) * hq],
                        )
                else:
                    nc.sync.dma_start(out=w13t, in_=w13_d[fb])

                gps = psg.tile([128, TC], f32, tag="gps")
                for kb in range(KB):
                    nc.tensor.matmul(
                        gps,
                        w13t[:, 0, kb * 128 : (kb + 1) * 128],
                        xtile[:, kb, :],
                        start=(kb == 0),
                        stop=(kb == KB - 1),
                    )
                ups = psu.tile([128, TC], f32, tag="ups")
                for kb in range(KB):
                    nc.tensor.matmul(
                        ups,
                        w13t[:, 1, kb * 128 : (kb + 1) * 128],
                        xtile[:, kb, :],
                        start=(kb == 0),
                        stop=(kb == KB - 1),
                    )

                # ACT evacuates both PSUM banks (Silu and Copy live in the
                # same ACT table, so alternating them reloads nothing); the
                # DVE multiply then depends on one engine only.
                sg = sgpool.tile([128, TC], f32, tag="sg")
                nc.scalar.activation(
                    sg, gps, mybir.ActivationFunctionType.Silu
                )
                usb = upool.tile([128, TC], f32, tag="usb")
                nc.scalar.copy(usb, ups)
                fut = fpool.tile(
                    [128, TC], bf16, tag="fused", name=f"fused{fb}"
                )
                nc.vector.tensor_tensor(
                    fut, sg, usb, mybir.AluOpType.mult
                )
                fused.append(fut)

            for hc in range(HCOLS):
                pss = []
                for tb in range(TB):
                    ps = psb.tile(
                        [128, 512], f32, tag="pss", name=f"pss{hc}_{tb}"
                    )
                    pss.append(ps)
                for j in range(FB2):
                    w2t = w2pool.tile([128, 2, 512], bf16, tag="w2t")
                    nc.sync.dma_start(out=w2t, in_=w2_d[hc, j])
                    for i in range(2):
                        fb = 2 * j + i
                        for tb in range(TB):
                            nc.tensor.matmul(
                                pss[tb],
                                fused[fb][:, tb * 128 : (tb + 1) * 128],
                                w2t[:, i, :],
                                start=(fb == 0),
                                stop=(fb == FB - 1),
                            )
                for tb in range(TB):
                    ot = opool.tile(
                        [128, 512], f32, tag="ot", name=f"ot{hc}_{tb}"
                    )
                    # Alternate DVE/ACT so the four evacuations drain in
                    # parallel; frees PSUM banks for the next hc sooner.
                    if tb % 2 == 0:
                        nc.vector.tensor_copy(ot, pss[tb])
                    else:
                        nc.scalar.copy(ot, pss[tb])
                    nc.sync.dma_start(
                        out=out_d[
                            tb * 128 : (tb + 1) * 128,
                            hc * 512 : (hc + 1) * 512,
                        ],
                        in_=ot,
                    )
    nc.finalize()
    return nc


def _dequant(wq, s):
    wq = np.asarray(wq, dtype=np.float32)
    s = np.asarray(s, dtype=np.float32)
    n, k = wq.shape
    nb, kb = s.shape
    w = wq.reshape(nb, n // nb, kb, k // kb) * s[:, None, :, None]
    return w.reshape(n, k)


def _prep_inputs(hidden_states, w1_q, w1_s, w3_q, w3_s, w2_q, w2_s):
    import ml_dtypes

    bf = ml_dtypes.bfloat16

    w1 = _dequant(w1_q, w1_s).astype(bf)  # [F, H]
    w3 = _dequant(w3_q, w3_s).astype(bf)  # [F, H]
    w2 = _dequant(w2_q, w2_s).astype(bf)  # [H, F]

    # w1p[fb, p, kb*128+c] = w1[fb*128+c, kb*128+p]  (and same for w3);
    # interleaved per partition: w13p[fb, p, 0] = w1 row, [fb, p, 1] = w3.
    w1p = w1.reshape(FB, 128, KB, 128).transpose(0, 3, 2, 1).reshape(FB, 128, H)
    w3p = w3.reshape(FB, 128, KB, 128).transpose(0, 3, 2, 1).reshape(FB, 128, H)
    w13p = np.ascontiguousarray(np.stack([w1p, w3p], axis=2))  # [FB,128,2,H]

    # w2p[hc, j, p, i, c] = w2[hc*512+c, (2j+i)*128+p]
    w2p = np.ascontiguousarray(
        np.asarray(w2).reshape(HCOLS, 512, FB2, 2, 128).transpose(0, 2, 4, 3, 1)
    )

    x = np.asarray(hidden_states, dtype=np.float32).astype(bf)
    xts = []
    for c in range(NCORES):
        xc = x[c * TC : (c + 1) * TC, :]
        # xt[p, kb, t] = xc[t, kb*128+p] — partition-major, so the whole
        # 2MB x-transpose lands in one DMA with 16KB/partition contiguous.
        xts.append(
            np.ascontiguousarray(xc.reshape(TC, KB, 128).transpose(2, 1, 0))
        )

    return [
        {"xt": xts[c], "w13p": w13p, "w2p": w2p}
        for c in range(NCORES)
    ]


def _run(in_maps, **kwargs):
    from concourse.bass_utils import run_bass_kernel_spmd

    if "nc" not in _CACHE:
        _CACHE["nc"] = _build_program()
    res = run_bass_kernel_spmd(
        _CACHE["nc"], in_maps, list(range(NCORES)), **kwargs
    )
    out = np.concatenate(
        [res.results[c]["out"] for c in range(NCORES)], axis=0
    )
    return np.asarray(out, dtype=np.float32), res


def kernel(hidden_states, w1_q, w1_s, w3_q, w3_s, w2_q, w2_s):
    in_maps = _prep_inputs(
        hidden_states, w1_q, w1_s, w3_q, w3_s, w2_q, w2_s
    )
    out, _ = _run(in_maps)
    return out



# revision 2
# speedup vs baseline: 1.0008x; 1.0008x over previous
"""Fused FP8-block-quantized MLP (silu(x@w1.T) * (x@w3.T)) @ w2.T on 8 trn2 cores.

Sharding: data-parallel over tokens. Each core gets T/8 = 512 tokens and the
full (dequantized, bf16) weights; there are no collectives. Host-side prep
dequantizes the block-quantized weights, casts to bf16, and lays tensors out
partition-major so every device DMA is one large contiguous transfer.

Device kernel per core (all matmuls bf16, fp32 PSUM accumulation):
  warmup:  a chain of dummy matmuls on a memset tile runs during the initial
           DMA wait so the PE HAM clock-gate reaches 8/8 before real work.
  phase A: for each 128-row block fb of F: g.T/u.T [128f, 512t] accumulated
           over 16 k-blocks of H; silu+copy on ACT, mul on DVE -> fusedT
           kept in SBUF.
  phase B: out [512t, 2048h] = fusedT.T @ w2.T, streaming w2 column blocks,
           accumulating over the 56 f-blocks in PSUM. Output stored bf16.
"""

import sys

import numpy as np

_REPO = "/opt/trn_rl_repo"
if _REPO not in sys.path:
    sys.path.insert(0, _REPO)

T, H, F = 4096, 2048, 7168
NCORES = 8
TC = T // NCORES      # 512 tokens per core
KB = H // 128         # 16 contraction blocks for matmul 1/3
FB = F // 128         # 56 f blocks
FB2 = FB // 2         # w2 blocks are streamed in pairs
HCOLS = H // 512      # 4 output column groups
TB = TC // 128        # 4 token blocks
NWARM = 160           # dummy matmuls to warm the PE clock gate

_CACHE = {}


def _build_program():
    import concourse.mybir as mybir
    from concourse import bacc
    from concourse.tile import TileContext

    bf16 = mybir.dt.bfloat16
    f32 = mybir.dt.float32

    # Bacc (not bass.Bass): its finalize() runs generate_event_semaphores,
    # which splits multi-wait sync_info into EventSemaphore instructions —
    # TRN2 instructions physically carry at most one sem wait.
    nc = bacc.Bacc()
    # All inputs are laid out partition-major on the host so each DMA below
    # is a single large transfer with contiguous per-partition rows.
    xt_d = nc.declare_dram_parameter("xt", [128, KB, TC], bf16, isOutput=False)
    w13_d = nc.declare_dram_parameter(
        "w13p", [FB, 128, 2, H], bf16, isOutput=False
    )
    w2_d = nc.declare_dram_parameter(
        "w2p", [HCOLS, FB2, 128, 2, 512], bf16, isOutput=False
    )
    out_d = nc.declare_dram_parameter("out", [TC, H], bf16, isOutput=True)

    with TileContext(nc) as tc:
        with (
            tc.tile_pool(name="xpool", bufs=1) as xpool,
            tc.tile_pool(name="wpool", bufs=2) as wpool,
            tc.tile_pool(name="w2pool", bufs=8) as w2pool,
            tc.tile_pool(name="sgpool", bufs=3) as sgpool,
            tc.tile_pool(name="upool", bufs=3) as upool,
            tc.tile_pool(name="fpool", bufs=FB) as fpool,
            tc.tile_pool(name="opool", bufs=HCOLS * TB) as opool,
        ):
            xtile = xpool.tile([128, KB, TC], bf16)

            fused = []
            with (
                tc.tile_pool(name="psg", bufs=3, space="PSUM") as psg,
                tc.tile_pool(name="psu", bufs=3, space="PSUM") as psu,
                tc.tile_pool(name="psw", bufs=1, space="PSUM") as psw,
            ):
                # Warm up the PE HAM clock gate during the startup DMA wait:
                # a long chain of self-contained matmuls on a zeroed tile.
                # ~3.4us of sustained PE activity flips the clock to 8/8, so
                # the real matmuls below start at full rate.
                warm = xpool.tile([128, 128], bf16, name="warm")
                nc.vector.memset(warm, 0.0)
                wps = psw.tile([128, 128], f32, name="warmps")
                for i in range(NWARM):
                    nc.tensor.matmul(
                        wps, warm, warm,
                        start=(i == 0), stop=(i == NWARM - 1),
                    )

                for fb in range(FB):
                    w13t = wpool.tile([128, 2, H], bf16, tag="w13t")
                    if fb == 0:
                        # Front-load exactly what the first matmuls consume:
                        # kb=0 of x plus the first w13 column block land
                        # first, so the PE starts ~4us earlier than waiting
                        # for full-tile transfers.
                        nc.sync.dma_start(
                            out=xtile[:, 0:1, :], in_=xt_d[:, 0:1, :]
                        )
                        nc.sync.dma_start(
                            out=w13t[:, :, 0:128], in_=w13_d[fb][:, :, 0:128]
                        )
                        nc.sync.dma_start(
                            out=xtile[:, 1:4, :], in_=xt_d[:, 1:4, :]
                        )
                        nc.sync.dma_start(
                            out=w13t[:, :, 128:512],
                            in_=w13_d[fb][:, :, 128:512],
                        )
                        nc.sync.dma_start(
                            out=xtile[:, 4:8, :], in_=xt_d[:, 4:8, :]
                        )
                        nc.sync.dma_start(
                            out=w13t[:, :, 512:1024],
                            in_=w13_d[fb][:, :, 512:1024],
                        )
                        nc.sync.dma_start(
                            out=xtile[:, 8:16, :], in_=xt_d[:, 8:16, :]
                        )
                        nc.sync.dma_start(
                            out=w13t[:, :, 1024:2048],
                            in_=w13_d[fb][:, :, 1024:2048],
                        )
                    else:
                        nc.sync.dma_start(out=w13t, in_=w13_d[fb])

                    gps = psg.tile([128, TC], f32, tag="gps")
                    for kb in range(KB):
                        nc.tensor.matmul(
                            gps,
                            w13t[:, 0, kb * 128 : (kb + 1) * 128],
                            xtile[:, kb, :],
                            start=(kb == 0),
                            stop=(kb == KB - 1),
                        )
                    ups = psu.tile([128, TC], f32, tag="ups")
                    for kb in range(KB):
                        nc.tensor.matmul(
                            ups,
                            w13t[:, 1, kb * 128 : (kb + 1) * 128],
                            xtile[:, kb, :],
                            start=(kb == 0),
                            stop=(kb == KB - 1),
                        )

                    # ACT evacuates both PSUM banks (Silu and Copy live in
                    # the same ACT table, so alternating them reloads
                    # nothing); the DVE multiply then depends on one engine.
                    sg = sgpool.tile([128, TC], f32, tag="sg")
                    nc.scalar.activation(
                        sg, gps, mybir.ActivationFunctionType.Silu
                    )
                    usb = upool.tile([128, TC], f32, tag="usb")
                    nc.scalar.copy(usb, ups)
                    fut = fpool.tile(
                        [128, TC], bf16, tag="fused", name=f"fused{fb}"
                    )
                    nc.vector.tensor_tensor(
                        fut, sg, usb, mybir.AluOpType.mult
                    )
                    fused.append(fut)

            # Phase A PSUM pools are closed: phase B gets all 8 banks, so
            # consecutive hc accumulation groups never wait on evacuation.
            with tc.tile_pool(name="psb", bufs=8, space="PSUM") as psb:
                for hc in range(HCOLS):
                    pss = []
                    for tb in range(TB):
                        ps = psb.tile(
                            [128, 512], f32, tag="pss", name=f"pss{hc}_{tb}"
                        )
                        pss.append(ps)
                    for j in range(FB2):
                        w2t = w2pool.tile([128, 2, 512], bf16, tag="w2t")
                        nc.sync.dma_start(out=w2t, in_=w2_d[hc, j])
                        for i in range(2):
                            fb = 2 * j + i
                            for tb in range(TB):
                                nc.tensor.matmul(
                                    pss[tb],
                                    fused[fb][:, tb * 128 : (tb + 1) * 128],
                                    w2t[:, i, :],
                                    start=(fb == 0),
                                    stop=(fb == FB - 1),
                                )
                    for tb in range(TB):
                        ot = opool.tile(
                            [128, 512], bf16, tag="ot", name=f"ot{hc}_{tb}"
                        )
                        # Alternate DVE/ACT so the four evacuations drain in
                        # parallel; frees PSUM banks for the next hc sooner.
                        if tb % 2 == 0:
                            nc.vector.tensor_copy(ot, pss[tb])
                        else:
                            nc.scalar.copy(ot, pss[tb])
                        nc.sync.dma_start(
                            out=out_d[
                                tb * 128 : (tb + 1) * 128,
                                hc * 512 : (hc + 1) * 512,
                            ],
                            in_=ot,
                        )
    nc.finalize()
    return nc


def _dequant(wq, s):
    wq = np.asarray(wq, dtype=np.float32)
    s = np.asarray(s, dtype=np.float32)
    n, k = wq.shape
    nb, kb = s.shape
    w = wq.reshape(nb, n // nb, kb, k // kb) * s[:, None, :, None]
    return w.reshape(n, k)


def _prep_inputs(hidden_states, w1_q, w1_s, w3_q, w3_s, w2_q, w2_s):
    import ml_dtypes

    bf = ml_dtypes.bfloat16

    w1 = _dequant(w1_q, w1_s).astype(bf)  # [F, H]
    w3 = _dequant(w3_q, w3_s).astype(bf)  # [F, H]
    w2 = _dequant(w2_q, w2_s).astype(bf)  # [H, F]

    # w1p[fb, p, kb*128+c] = w1[fb*128+c, kb*128+p]  (and same for w3);
    # interleaved per partition: w13p[fb, p, 0] = w1 row, [fb, p, 1] = w3.
    w1p = w1.reshape(FB, 128, KB, 128).transpose(0, 3, 2, 1).reshape(FB, 128, H)
    w3p = w3.reshape(FB, 128, KB, 128).transpose(0, 3, 2, 1).reshape(FB, 128, H)
    w13p = np.ascontiguousarray(np.stack([w1p, w3p], axis=2))  # [FB,128,2,H]

    # w2p[hc, j, p, i, c] = w2[hc*512+c, (2j+i)*128+p]
    w2p = np.ascontiguousarray(
        np.asarray(w2).reshape(HCOLS, 512, FB2, 2, 128).transpose(0, 2, 4, 3, 1)
    )

    x = np.asarray(hidden_states, dtype=np.float32).astype(bf)
    xts = []
    for c in range(NCORES):
        xc = x[c * TC : (c + 1) * TC, :]
        # xt[p, kb, t] = xc[t, kb*128+p] — partition-major, so the whole
        # 2MB x-transpose lands in one DMA with 16KB/partition contiguous.
        xts.append(
            np.ascontiguousarray(xc.reshape(TC, KB, 128).transpose(2, 1, 0))
        )

    return [
        {"xt": xts[c], "w13p": w13p, "w2p": w2p}
        for c in range(NCORES)
    ]


def _run(in_maps, **kwargs):
    from concourse.bass_utils import run_bass_kernel_spmd

    if "nc" not in _CACHE:
        _CACHE["nc"] = _build_program()
    res = run_bass_kernel_spmd(
        _CACHE["nc"], in_maps, list(range(NCORES)), **kwargs
    )
    out = np.concatenate(
        [res.results[c]["out"] for c in range(NCORES)], axis=0
    )
    return np.asarray(out).astype(np.float32), res


def kernel(hidden_states, w1_q, w1_s, w3_q, w3_s, w2_q, w2_s):
    in_maps = _prep_inputs(
        hidden_states, w1_q, w1_s, w3_q, w3_s, w2_q, w2_s
    )
    out, _ = _run(in_maps)
    return out


# revision 7
# speedup vs baseline: 1.0027x; 1.0019x over previous
"""Fused FP8-block-quantized MLP (silu(x@w1.T) * (x@w3.T)) @ w2.T on 8 trn2 cores.

Sharding: data-parallel over tokens. Each core gets T/8 = 512 tokens and the
full (dequantized, bf16) weights; there are no collectives. Host-side prep
dequantizes the block-quantized weights, casts to bf16, and lays tensors out
partition-major so every device DMA is one large contiguous transfer.

Device kernel per core (all matmuls bf16, fp32 PSUM accumulation):
  warmup:  a chain of dummy matmuls on a memset tile runs during the initial
           DMA wait so the PE HAM clock-gate reaches 8/8 before real work.
  phase A: for each 128-row block fb of F: g.T/u.T [128f, 512t] accumulated
           over 16 k-blocks of H; silu+copy on ACT, mul on DVE -> fusedT
           kept in SBUF.
  phase B: out [512t, 2048h] = fusedT.T @ w2.T, streaming w2 column blocks,
           accumulating over the 56 f-blocks in PSUM. Output stored bf16.
"""

import sys

import numpy as np

_REPO = "/opt/trn_rl_repo"
if _REPO not in sys.path:
    sys.path.insert(0, _REPO)

T, H, F = 4096, 2048, 7168
NCORES = 8
TC = T // NCORES      # 512 tokens per core
KB = H // 128         # 16 contraction blocks for matmul 1/3
FB = F // 128         # 56 f blocks
FB2 = FB // 2         # w2 blocks are streamed in pairs
HCOLS = H // 512      # 4 output column groups
TB = TC // 128        # 4 token blocks
NWARM = 40            # dummy matmuls to warm the PE clock gate

_CACHE = {}


def _build_program():
    import concourse.mybir as mybir
    from concourse import bacc
    from concourse.tile import TileContext

    bf16 = mybir.dt.bfloat16
    f32 = mybir.dt.float32

    # Bacc (not bass.Bass): its finalize() runs generate_event_semaphores,
    # which splits multi-wait sync_info into EventSemaphore instructions —
    # TRN2 instructions physically carry at most one sem wait.
    nc = bacc.Bacc()
    # All inputs are laid out partition-major on the host so each DMA below
    # is a single large transfer with contiguous per-partition rows.
    xt_d = nc.declare_dram_parameter("xt", [128, KB, TC], bf16, isOutput=False)
    w13_d = nc.declare_dram_parameter(
        "w13p", [FB, 128, 2, H], bf16, isOutput=False
    )
    w2_d = nc.declare_dram_parameter(
        "w2p", [HCOLS, FB2, 128, 2, 512], bf16, isOutput=False
    )
    # out[tb, p, hc, c] = result row tb*128+p, col hc*512+c; the host
    # reshape back to [TC, H] is free since the axes are already ordered.
    out_d = nc.declare_dram_parameter(
        "out", [TB, 128, HCOLS, 512], bf16, isOutput=True
    )

    with TileContext(nc) as tc:
        with (
            tc.tile_pool(name="xpool", bufs=1) as xpool,
            tc.tile_pool(name="wpool", bufs=2) as wpool,
            tc.tile_pool(name="w2pool", bufs=8) as w2pool,
            tc.tile_pool(name="sgpool", bufs=3) as sgpool,
            tc.tile_pool(name="upool", bufs=3) as upool,
            tc.tile_pool(name="fpool", bufs=FB) as fpool,
            tc.tile_pool(name="opool", bufs=HCOLS * TB) as opool,
        ):
            xtile = xpool.tile([128, KB, TC], bf16)

            fused = []
            with (
                tc.tile_pool(name="psg", bufs=3, space="PSUM") as psg,
                tc.tile_pool(name="psu", bufs=3, space="PSUM") as psu,
                tc.tile_pool(name="psw", bufs=1, space="PSUM") as psw,
            ):
                # Warm up the PE HAM clock gate during the startup DMA wait:
                # a long chain of self-contained matmuls on a zeroed tile.
                # ~3.4us of sustained PE activity flips the clock to 8/8, so
                # the real matmuls below start at full rate.
                warm = xpool.tile([128, 128], bf16, name="warm")
                nc.vector.memset(warm, 0.0)
                wps = psw.tile([128, 128], f32, name="warmps")
                for i in range(NWARM):
                    nc.tensor.matmul(
                        wps, warm, warm,
                        start=(i == 0), stop=(i == NWARM - 1),
                    )

                for fb in range(FB):
                    w13t = wpool.tile([128, 2, H], bf16, tag="w13t")
                    if fb == 0:
                        # Front-load exactly what the first matmuls consume:
                        # kb=0 of x plus the first w13 column block land
                        # first, so the PE starts ~4us earlier than waiting
                        # for full-tile transfers.
                        nc.sync.dma_start(
                            out=xtile[:, 0:1, :], in_=xt_d[:, 0:1, :]
                        )
                        nc.sync.dma_start(
                            out=w13t[:, :, 0:128], in_=w13_d[fb][:, :, 0:128]
                        )
                        nc.sync.dma_start(
                            out=xtile[:, 1:4, :], in_=xt_d[:, 1:4, :]
                        )
                        nc.sync.dma_start(
                            out=w13t[:, :, 128:512],
                            in_=w13_d[fb][:, :, 128:512],
                        )
                        nc.sync.dma_start(
                            out=xtile[:, 4:8, :], in_=xt_d[:, 4:8, :]
                        )
                        nc.sync.dma_start(
                            out=w13t[:, :, 512:1024],
                            in_=w13_d[fb][:, :, 512:1024],
                        )
                        nc.sync.dma_start(
                            out=xtile[:, 8:16, :], in_=xt_d[:, 8:16, :]
                        )
                        nc.sync.dma_start(
                            out=w13t[:, :, 1024:2048],
                            in_=w13_d[fb][:, :, 1024:2048],
                        )
                    else:
                        nc.sync.dma_start(out=w13t, in_=w13_d[fb])

                    gps = psg.tile([128, TC], f32, tag="gps")
                    for kb in range(KB):
                        nc.tensor.matmul(
                            gps,
                            w13t[:, 0, kb * 128 : (kb + 1) * 128],
                            xtile[:, kb, :],
                            start=(kb == 0),
                            stop=(kb == KB - 1),
                        )
                    ups = psu.tile([128, TC], f32, tag="ups")
                    for kb in range(KB):
                        nc.tensor.matmul(
                            ups,
                            w13t[:, 1, kb * 128 : (kb + 1) * 128],
                            xtile[:, kb, :],
                            start=(kb == 0),
                            stop=(kb == KB - 1),
                        )

                    # ACT evacuates both PSUM banks (Silu and Copy live in
                    # the same ACT table, so alternating them reloads
                    # nothing); the DVE multiply then depends on one engine.
                    sg = sgpool.tile([128, TC], f32, tag="sg")
                    nc.scalar.activation(
                        sg, gps, mybir.ActivationFunctionType.Silu
                    )
                    usb = upool.tile([128, TC], f32, tag="usb")
                    nc.scalar.copy(usb, ups)
                    fut = fpool.tile(
                        [128, TC], bf16, tag="fused", name=f"fused{fb}"
                    )
                    nc.vector.tensor_tensor(
                        fut, sg, usb, mybir.AluOpType.mult
                    )
                    fused.append(fut)

            # Phase A PSUM pools are closed: phase B gets all 8 banks as two
            # 4-bank tiles that alternate per hc, so consecutive hc
            # accumulation groups never wait on evacuation. One tile holds
            # all four token blocks -> one evacuation pass per engine and a
            # single output DMA trigger per hc (each DMA_DIRECT2D costs
            # ~630ns on the serial Sync engine — 4 of them dominated the
            # kernel tail).
            with tc.tile_pool(name="psb", bufs=2, space="PSUM") as psb:
                for hc in range(HCOLS):
                    pst = psb.tile([128, TB, 512], f32, tag="pss",
                                   name=f"pss{hc}")
                    for j in range(FB2):
                        w2t = w2pool.tile([128, 2, 512], bf16, tag="w2t")
                        nc.sync.dma_start(out=w2t, in_=w2_d[hc, j])
                        for i in range(2):
                            fb = 2 * j + i
                            for tb in range(TB):
                                nc.tensor.matmul(
                                    pst[:, tb, :],
                                    fused[fb][:, tb * 128 : (tb + 1) * 128],
                                    w2t[:, i, :],
                                    start=(fb == 0),
                                    stop=(fb == FB - 1),
                                )
                    ot = opool.tile(
                        [128, TB, 512], bf16, tag="ot", name=f"ot{hc}"
                    )
                    # Split the evacuation between DVE and ACT so both
                    # halves drain in parallel.
                    nc.vector.tensor_copy(ot[:, 0:2, :], pst[:, 0:2, :])
                    nc.scalar.copy(ot[:, 2:4, :], pst[:, 2:4, :])
                    # out_d[tb, p, hc, c] = ot[p, tb, c]
                    nc.sync.dma_start(
                        out=out_d[:, :, hc, :].rearrange("tb p c -> p tb c"),
                        in_=ot,
                    )
    nc.finalize()
    return nc


def _dequant(wq, s):
    wq = np.asarray(wq, dtype=np.float32)
    s = np.asarray(s, dtype=np.float32)
    n, k = wq.shape
    nb, kb = s.shape
    w = wq.reshape(nb, n // nb, kb, k // kb) * s[:, None, :, None]
    return w.reshape(n, k)


def _prep_inputs(hidden_states, w1_q, w1_s, w3_q, w3_s, w2_q, w2_s):
    import ml_dtypes

    bf = ml_dtypes.bfloat16

    w1 = _dequant(w1_q, w1_s).astype(bf)  # [F, H]
    w3 = _dequant(w3_q, w3_s).astype(bf)  # [F, H]
    w2 = _dequant(w2_q, w2_s).astype(bf)  # [H, F]

    # w1p[fb, p, kb*128+c] = w1[fb*128+c, kb*128+p]  (and same for w3);
    # interleaved per partition: w13p[fb, p, 0] = w1 row, [fb, p, 1] = w3.
    w1p = w1.reshape(FB, 128, KB, 128).transpose(0, 3, 2, 1).reshape(FB, 128, H)
    w3p = w3.reshape(FB, 128, KB, 128).transpose(0, 3, 2, 1).reshape(FB, 128, H)
    w13p = np.ascontiguousarray(np.stack([w1p, w3p], axis=2))  # [FB,128,2,H]

    # w2p[hc, j, p, i, c] = w2[hc*512+c, (2j+i)*128+p]
    w2p = np.ascontiguousarray(
        np.asarray(w2).reshape(HCOLS, 512, FB2, 2, 128).transpose(0, 2, 4, 3, 1)
    )

    x = np.asarray(hidden_states, dtype=np.float32).astype(bf)
    xts = []
    for c in range(NCORES):
        xc = x[c * TC : (c + 1) * TC, :]
        # xt[p, kb, t] = xc[t, kb*128+p] — partition-major, so the whole
        # 2MB x-transpose lands in one DMA with 16KB/partition contiguous.
        xts.append(
            np.ascontiguousarray(xc.reshape(TC, KB, 128).transpose(2, 1, 0))
        )

    return [
        {"xt": xts[c], "w13p": w13p, "w2p": w2p}
        for c in range(NCORES)
    ]


def _run(in_maps, **kwargs):
    from concourse.bass_utils import run_bass_kernel_spmd

    if "nc" not in _CACHE:
        _CACHE["nc"] = _build_program()
    res = run_bass_kernel_spmd(
        _CACHE["nc"], in_maps, list(range(NCORES)), **kwargs
    )
    out = np.concatenate(
        [np.asarray(res.results[c]["out"]).reshape(TC, H) for c in range(NCORES)],
        axis=0,
    )
    return out.astype(np.float32), res


def kernel(hidden_states, w1_q, w1_s, w3_q, w3_s, w2_q, w2_s):
    in_maps = _prep_inputs(
        hidden_states, w1_q, w1_s, w3_q, w3_s, w2_q, w2_s
    )
    out, _ = _run(in_maps)
    return out
